# revision 23
# baseline (speedup 1.0000x reference)
"""Paged decode attention (nn_Attention_5626407157951) on 8 Trainium2 cores.

Tensor-parallel over heads: each core owns 4 of 32 heads. Per core:
  qkv = hidden @ W_pack[:, own cols]      (bf16 matmuls, fp32 acc)
  rotary(q, k) at pos=hist                (DVE, fp32; host-built cos/sin)
  scores_T[s, (h,pair)] = K_tile^T q      (PE, K stationary, q moving, bf16)
  softmax without max-subtraction; new token handled analytically:
      out = (sum_s exp(s)*v_s + e_new*v_new) / (sum_s exp(s) + e_new)
  out_partial = attn @ o_proj[:, own dims].T ; host sums the 8 partials.

Everything DMA'd is bf16 (tolerance is 2e-2; bf16 end-to-end lands ~1e-3).
KV is host-packed per request (only valid 128-token pairs), contiguous in
DRAM so each request is one large DMA with multi-KB per-partition runs.
"""

import math
import os

import ml_dtypes
import numpy as np

import concourse.bass as bass
import concourse.mybir as mybir
import concourse.tile as tile
from concourse.bass_utils import run_bass_kernel_spmd
from concourse.vector_clock import ScopedClock

B = 32          # batch (decode requests)
H = 32          # total heads
HL = 4          # heads per core
D = 128         # head dim
HID = 4096
BS = 64         # cache block size
NBLK = 16       # blocks per request
NCORES = 8
KT = HID // 128         # 32 contraction tiles for qkv proj
PAIRS = NBLK // 2       # 8 block-pairs (128 tokens each) per request
ROPE_BASE = 10000.0
KTB = 2                 # W_pack kt tiles fetched per DMA

F32 = mybir.dt.float32
BF = mybir.dt.bfloat16
BF_NP = ml_dtypes.bfloat16
EXP_FN = mybir.ActivationFunctionType.Exp
MUL = mybir.AluOpType.mult
ADD = mybir.AluOpType.add
SUB = mybir.AluOpType.subtract

LAST_RESULTS = None  # test harness peeks at this for profiling info

# ---------------------------------------------------------------------------
# This walrus build accepts very few sync-waits per instruction; the Tile
# kernel-tail drain accumulates one wait per sem lane. Split the waits over
# several drain instructions (all before the barrier, so semantics hold).
_MAX_DRAIN_WAITS = 1


def _patched_drain_and_barrier(self, tick_clock, wait_clock):
    nc = self.nc
    drain_inst = nc.sync.drain()
    wait_clock.add_sem_waits(
        drain_inst.ins, ScopedClock({None: tick_clock.global_clock})
    )
    si = drain_inst.ins.sync_info
    if si is not None and si.on_wait and len(si.on_wait) > _MAX_DRAIN_WAITS:
        waits = list(si.on_wait)
        drain_inst.ins.sync_info = mybir.SyncInfo(
            on_wait=waits[:_MAX_DRAIN_WAITS], on_update=list(si.on_update or [])
        )
        rest = waits[_MAX_DRAIN_WAITS:]
        for i in range(0, len(rest), _MAX_DRAIN_WAITS):
            extra = nc.sync.drain()
            extra.ins.sync_info = mybir.SyncInfo(
                on_wait=rest[i : i + _MAX_DRAIN_WAITS], on_update=[]
            )
    nc.all_engine_barrier()
    popped = nc._tile_sem_poison_stack.pop()
    assert popped is self._sem_poison
    nc.clear_and_free_semaphores(list(self.sems.allocated().values()))
    nc.all_engine_barrier()


tile.TileContext._drain_and_barrier = _patched_drain_and_barrier


def _split_excess_waits(nc, limit=1):
    """Walrus rejects instructions carrying more than ~1 sync wait. Hoist the
    excess onto NoOps inserted just before, on the same engine queue (the
    queue blocks on them first, so semantics are identical)."""
    for fn in nc.m.functions:
        for bb in fn.blocks:
            out = []
            changed = False
            for inst in list(bb.instructions):
                si = getattr(inst, "sync_info", None)
                if si is not None and si.on_wait and len(si.on_wait) > limit:
                    waits = list(si.on_wait)
                    extra, keep = waits[:-limit], waits[-limit:]
                    for i in range(0, len(extra), limit):
                        nop = mybir.InstNoOp(
                            name=nc.get_next_instruction_name(),
                            ins=[], outs=[], engine=inst.engine,
                            sync_info=mybir.SyncInfo(
                                on_wait=extra[i : i + limit], on_update=[]
                            ),
                        )
                        nc.register_instruction(nop)
                        out.append(nop)
                    inst.sync_info = mybir.SyncInfo(
                        on_wait=keep, on_update=list(si.on_update or [])
                    )
                    changed = True
                out.append(inst)
            if changed:
                bb.instructions = out
# ---------------------------------------------------------------------------


def _build_nc(pairs):
    """Build the SPMD bass module. `pairs[b]` = number of 128-token cached
    pairs for request b (same on every core; head split is via input data)."""
    nc = bass.Bass()

    pairs, s16s = pairs
    kcols = [HL * s for s in s16s]            # exact-packed K cols per request
    vcols = [p * 512 for p in pairs]          # 128-padded V cols per request
    offs = np.concatenate([[0], np.cumsum([k + v for k, v in zip(kcols, vcols)])])
    G = int(offs[-1])  # total packed KV columns (per 128-partition row)

    def param(name, shape, dt):
        return nc.declare_dram_parameter(name, list(shape), dt, isOutput=False)

    hT = param("hT", [128, KT, B], BF)
    wp = param("wp", [128, KT, 3 * HL * D], BF)
    wo = param("wo", [HL, 128, HID], BF)
    kv = param("kv", [128, max(G, 1)], BF)   # per request: [K (h,s16) | V (p,h,d)]
    cs = param("cs", [B, 4 * HL * D], F32)
    maskp = param("mask", [128, B, HL, PAIRS], BF)   # multiplicative 0/1
    identp = param("ident", [B, B], F32)
    out_part = nc.declare_dram_parameter("out_part", [B, HID], F32, isOutput=True)

    HD = HL * D  # 512 local attention dims

    with tile.TileContext(nc) as tc:
        with (
            tc.tile_pool(name="const", bufs=1) as cpool,
            tc.tile_pool(name="work", bufs=1) as wpool,
            tc.tile_pool(name="wtiles", bufs=8) as wtp,
            tc.tile_pool(name="kv", bufs=7) as kvp,
            tc.tile_pool(name="small", bufs=4) as smp,
        ):
            # ---- constants ----
            ident = cpool.tile([B, B], F32)
            nc.scalar.dma_start(out=ident[:], in_=identp[:])
            ones = cpool.tile([128, 1], BF)
            nc.vector.memset(ones[:], 1.0)
            onesf = cpool.tile([1, HL * B], F32)
            nc.vector.memset(onesf[:], 1.0)
            mask_sb = cpool.tile([128, B, HL, PAIRS], BF)
            nc.scalar.dma_start(out=mask_sb[:], in_=maskp[:])
            cs_sb = cpool.tile([B, 4 * HD], F32)
            nc.scalar.dma_start(out=cs_sb[:], in_=cs[:])
            hT_sb = cpool.tile([128, KT, B], BF)
            nc.scalar.dma_start(out=hT_sb[:], in_=hT[:])

            # W_pack chunks issued ahead of the KV prefetch (same queue) so
            # the wire strictly prioritizes the qkv phase's critical input;
            # tiny leading chunks let the first matmuls start sooner
            wp_chunks = [(0, 1), (1, 2)] + [(k, k + 2) for k in range(2, KT, 2)]
            wp_tiles = {}

            def load_wp(c):
                a, e = wp_chunks[c]
                wpt = wtp.tile([128, KTB, 3 * HD], BF, tag="wpt")
                nc.sync.dma_start(out=wpt[:, 0 : e - a, :], in_=wp[:, a:e, :])
                wp_tiles[c] = wpt

            for c in range(len(wp_chunks)):
                load_wp(c)

            # per-request KV loads (one DMA per tensor per request)
            kv_tiles = {}

            def load_b(b):
                o = int(offs[b])
                w = int(offs[b + 1]) - o
                t = kvp.tile([128, w], BF, tag="kv")
                nc.sync.dma_start(out=t[:], in_=kv[:, o : o + w])
                kv_tiles[b] = t

            for b in sorted(range(B), key=lambda b: -pairs[b]):
                if pairs[b] > 0:
                    load_b(b)
                    if len(kv_tiles) >= 7:
                        break

            # accumulators written per-b, read in the epilogue
            atsb = wpool.tile([128, HL * B], F32)   # cached attn, col h*32+b
            nc.vector.memset(atsb[:], 0.0)
            dnm = wpool.tile([1, HL * B], F32)      # cached denom, col h*32+b
            nc.vector.memset(dnm[:], 0.0)

            with tc.tile_pool(name="psA", bufs=1, space="PSUM") as psA:
                # PE warmup transpose so `ident` is observed by PE before the
                # real (fp32, single-wait-slot) transposes below.
                tp0 = psA.tile([B, B], F32, tag="tp0")
                nc.tensor.transpose(tp0[:], ident[:], ident[:])

                # ---- phase 1: qkv = hidden @ W_pack (bf16) ----
                with nc.named_scope("qkv"):
                    qkv_ps = psA.tile([B, 3 * HD], F32, tag="qkv")
                    for c, (a, e) in enumerate(wp_chunks):
                        wpt = wp_tiles.pop(c)
                        for kj in range(e - a):
                            kt = a + kj
                            for n in range(3):
                                nc.tensor.matmul(
                                    qkv_ps[:, n * HD : (n + 1) * HD],
                                    hT_sb[:, kt, :],
                                    wpt[:, kj, n * HD : (n + 1) * HD],
                                    start=(kt == 0),
                                    stop=(kt == KT - 1),
                                )

                    qkv_sb = wpool.tile([B, 3 * HD], F32)
                    nc.vector.tensor_copy(qkv_sb[:], qkv_ps[:])

                # ---- phase 2: rotary (fp32, DVE) + transposes ----
                with nc.named_scope("rope"):
                    def rope(src_off, cs_off):
                        src = qkv_sb[:, src_off : src_off + HD]
                        t1 = wpool.tile([B, HD], F32, tag="rope_t1")
                        nc.vector.tensor_tensor(
                            t1[:], src, cs_sb[:, cs_off : cs_off + HD], MUL
                        )
                        sh = wpool.tile([B, HD], F32, tag="rope_sh")
                        sh4 = sh[:].rearrange("b (h d) -> b h d", h=HL)
                        sr4 = qkv_sb[:, src_off : src_off + HD].rearrange(
                            "b (h d) -> b h d", h=HL
                        )
                        cs2 = cs_sb[:, cs_off + HD : cs_off + 2 * HD].rearrange(
                            "b (h d) -> b h d", h=HL
                        )
                        nc.vector.tensor_tensor(
                            sh4[:, :, 0:64], sr4[:, :, 64:128], cs2[:, :, 0:64], MUL
                        )
                        nc.vector.tensor_tensor(
                            sh4[:, :, 64:128], sr4[:, :, 0:64], cs2[:, :, 64:128], MUL
                        )
                        nc.vector.tensor_tensor(
                            qkv_sb[:, src_off : src_off + HD], t1[:], sh[:], ADD
                        )

                    rope(0, 0)          # q (scale folded into tables)
                    rope(HD, 2 * HD)    # k

                    # PE transposes -> [128(d), (h,b)] fp32 tiles
                    qT = wpool.tile([128, HL * B], F32)
                    kT = wpool.tile([128, HL * B], F32)
                    vT = wpool.tile([128, HL * B], F32)
                    for off, dst in ((0, qT), (HD, kT), (2 * HD, vT)):
                        for h in range(HL):
                            tp = psA.tile([128, B], F32, tag="tp")
                            inp = qkv_sb[:, off + h * D : off + (h + 1) * D]
                            nc.tensor.transpose(tp[:], inp, ident[:])
                            nc.vector.tensor_copy(dst[:, h * B : (h + 1) * B], tp[:])

                    qT_bf = wpool.tile([128, HL * B], BF)
                    nc.vector.tensor_copy(qT_bf[:], qT[:])

                    # new-token scores: e_new[(h,b)] = exp(q . k_new)
                    prod = wpool.tile([128, HL * B], F32)
                    nc.vector.tensor_tensor(prod[:], qT[:], kT[:], MUL)
                    prod_bf = wpool.tile([128, HL * B], BF)
                    nc.vector.tensor_copy(prod_bf[:], prod[:])
                    sn_ps = psA.tile([1, HL * B], F32, tag="sn")
                    nc.tensor.matmul(sn_ps[:], ones[:], prod_bf[:], start=True, stop=True)
                    e_new = wpool.tile([1, HL * B], F32)
                    nc.scalar.activation(e_new[:], sn_ps[:], EXP_FN)

            # ---- phase 3: per-request paged attention ----
            # o_proj weight DMAs are interleaved into the attention tail so
            # they fill the wire without delaying critical-path KV loads
            wo_tiles = {}
            border = sorted(range(B), key=lambda b: -pairs[b])
            wo_sched = {24: 0, 27: 1, 29: 2, 31: 3}   # loop positions

            def issue_wo(h):
                woh = kvp.tile([128, HID], BF, tag="kv")
                nc.scalar.dma_start(out=woh[:], in_=wo[h])
                wo_tiles[h] = woh

            with (
                tc.tile_pool(name="psB", bufs=4, space="PSUM") as psB,
                tc.tile_pool(name="psBa", bufs=3, space="PSUM") as psBa,
                tc.tile_pool(name="psB2", bufs=1, space="PSUM") as psB2,
                nc.named_scope("attn"),
            ):
                for bi, b in enumerate(border):
                    if bi in wo_sched:
                        issue_wo(wo_sched[bi])
                    pb = pairs[b]
                    if pb == 0:
                        continue
                    if b not in kv_tiles:
                        load_b(b)
                    ni = bi + 1
                    loaded = sum(1 for t in kv_tiles if t != b)
                    while ni < B and loaded < 6:
                        nb = border[ni]
                        if pairs[nb] > 0 and nb not in kv_tiles:
                            load_b(nb)
                            loaded += 1
                        ni += 1
                    t = kv_tiles.pop(b)
                    s16 = s16s[b]
                    kt_b = t[:, 0 : HL * s16].rearrange("d (h s) -> d h s", h=HL)
                    vt_b = t[:, HL * s16 : HL * s16 + pb * 512].rearrange(
                        "s (p h d) -> s p h d", p=pb, h=HL
                    )

                    # scores^T: [128(s), (h, pair)]; tail rows beyond the
                    # packed K are memset to 0 (exp->1, masked to 0)
                    scp = psB.tile([128, HL, pb], F32, tag="scp")
                    wlast = s16 - (pb - 1) * 128
                    if wlast < 128:
                        nc.vector.memset(scp[:, :, pb - 1], 0.0)
                    # 64-col stationary halves land on different PE
                    # column groups, letting LDWEIGHTS of one half overlap
                    # the in-flight matmul of the other
                    for h in range(HL):
                        qh = qT_bf[:, h * B + b : h * B + b + 1]
                        for p in range(pb):
                            w = min(128, s16 - p * 128)
                            w1 = min(64, w)
                            nc.tensor.matmul(
                                scp[0:w1, h, p : p + 1],
                                kt_b[:, h, p * 128 : p * 128 + w1],
                                qh, start=True, stop=True,
                            )
                            if w > 64:
                                nc.tensor.matmul(
                                    scp[64:w, h, p : p + 1],
                                    kt_b[:, h, p * 128 + 64 : p * 128 + w],
                                    qh, start=True, stop=True,
                                )

                    # exp -> probs, multiplicative 0/1 mask folded into the
                    # bf16 downcast (invalid slots in the last pair -> 0)
                    expb = smp.tile([128, HL, pb], F32, tag="expb")
                    nc.scalar.activation(expb[:], scp[:], EXP_FN)
                    ph = smp.tile([128, HL, pb], BF, tag="ph")
                    nc.vector.tensor_tensor(
                        ph[:], expb[:], mask_sb[:, b, :, 0:pb], MUL
                    )

                    # attn^T[d, h] = sum_s p[s] * V[s, d]
                    atp = psBa.tile([128, HL], F32, tag="atp")
                    for h in range(HL):
                        for p in range(pb):
                            nc.tensor.matmul(
                                atp[0:64, h : h + 1],
                                vt_b[:, p, h, 0:64],
                                ph[:, h, p : p + 1],
                                start=(p == 0), stop=(p == pb - 1),
                            )
                            nc.tensor.matmul(
                                atp[64:128, h : h + 1],
                                vt_b[:, p, h, 64:128],
                                ph[:, h, p : p + 1],
                                start=(p == 0), stop=(p == pb - 1),
                            )
                    nc.vector.tensor_copy(
                        atsb[:].rearrange("d (h b2) -> d h b2", h=HL)[:, :, b], atp[:]
                    )

                    # denominators: column sums of probs
                    dsp = psB2.tile([1, HL * pb], F32, tag="dsp")
                    nc.tensor.matmul(
                        dsp[:], ones[:], ph[:].rearrange("s h p -> s (h p)"),
                        start=True, stop=True,
                    )
                    nc.vector.reduce_sum(
                        dnm[:].rearrange("o (h b2) -> o h b2", h=HL)[:, :, b],
                        dsp[:].rearrange("o (h p) -> o h p", h=HL),
                        axis=mybir.AxisListType.X,
                    )

            # ---- epilogue: add new token, normalize, project ----
            with nc.named_scope("oproj"):
                dtot = wpool.tile([1, HL * B], F32)
                nc.vector.tensor_tensor(dtot[:], dnm[:], e_new[:], ADD)
                rec = wpool.tile([1, HL * B], F32)
                nc.vector.reciprocal(rec[:], dtot[:])
                att = wpool.tile([128, HL * B], F32)
                with tc.tile_pool(name="psD", bufs=1, space="PSUM") as psD:
                    # broadcast rows across partitions via K=1 outer products
                    ebp = psD.tile([128, HL * B], F32, tag="ebp")
                    nc.tensor.matmul(ebp[:], onesf[:], e_new[:], start=True, stop=True)
                    rbp = psD.tile([128, HL * B], F32, tag="rbp")
                    nc.tensor.matmul(rbp[:], onesf[:], rec[:], start=True, stop=True)

                    nc.vector.tensor_tensor(att[:], vT[:], ebp[:], MUL)
                    nc.vector.tensor_tensor(att[:], att[:], atsb[:], ADD)
                    nc.vector.tensor_tensor(att[:], att[:], rbp[:], MUL)
                att_bf = wpool.tile([128, HL * B], BF)
                nc.vector.tensor_copy(att_bf[:], att[:])

                with tc.tile_pool(name="psC", bufs=3, space="PSUM") as psC:
                    for h in range(HL):
                        if h not in wo_tiles:
                            issue_wo(h)
                    for n in range(8):
                        opsn = psC.tile([B, 512], F32, tag="ops")
                        for h in range(HL):
                            nc.tensor.matmul(
                                opsn[:],
                                att_bf[:, h * B : (h + 1) * B],
                                wo_tiles[h][:, n * 512 : (n + 1) * 512],
                                start=(h == 0),
                                stop=(h == HL - 1),
                            )
                        outc = smp.tile([B, 512], F32, tag="outc")
                        nc.vector.tensor_copy(outc[:], opsn[:])
                        nc.scalar.dma_start(
                            out=out_part[:, n * 512 : (n + 1) * 512], in_=outc[:]
                        )

    _split_excess_waits(nc)
    return nc


def _host_prep(hidden, W_pack, o_proj_weight, k_cache, v_cache, hist, block_offsets):
    """Build the 8 per-core input maps (numpy only)."""
    hidden = np.asarray(hidden, np.float32)
    W_pack = np.asarray(W_pack, np.float32)
    o_proj_weight = np.asarray(o_proj_weight, np.float32)
    k_cache = np.asarray(k_cache, np.float32)
    v_cache = np.asarray(v_cache, np.float32)
    hist = np.asarray(hist, np.int64)
    block_offsets = np.asarray(block_offsets, np.int64)

    pairs = [int((h + 127) // 128) for h in hist]
    s16s = [int((h + 15) // 16 * 16) for h in hist]
    kcols = [HL * s for s in s16s]
    vcols = [p * 512 for p in pairs]
    offs = np.concatenate([[0], np.cumsum([k + v for k, v in zip(kcols, vcols)])])
    G = int(offs[-1])

    # rope tables, scale folded into the q tables
    inv_freq = 1.0 / (ROPE_BASE ** (np.arange(0, D, 2, dtype=np.float32) / D))
    ang = hist.astype(np.float32)[:, None] * inv_freq[None, :]        # [B, 64]
    cos128 = np.concatenate([np.cos(ang), np.cos(ang)], -1)           # [B, 128]
    sin128 = np.concatenate([np.sin(ang), np.sin(ang)], -1)
    sign = np.concatenate([-np.ones(64), np.ones(64)]).astype(np.float32)
    sc = 1.0 / math.sqrt(D)
    tile_h = lambda x: np.tile(x, (1, HL)).astype(np.float32)         # [B, 512]
    cs = np.concatenate(
        [tile_h(cos128 * sc), tile_h(sin128 * sign * sc),
         tile_h(cos128), tile_h(sin128 * sign)], -1,
    )                                                                 # [B, 2048]

    # multiplicative mask over loaded pairs: pos 128*p + s valid iff < hist
    s_idx = np.arange(128)[:, None, None]                             # s
    p_idx = np.arange(PAIRS)[None, None, :]                           # pair
    pos = p_idx * 128 + s_idx                                         # [128,1,8]
    valid = pos < hist[None, :, None]                                 # [128,B,8]
    mask = np.repeat(valid[:, :, None, :], HL, axis=2).astype(BF_NP)  # [128,B,4,8]

    hT = np.ascontiguousarray(hidden.T)                               # [4096, 32]
    hT_bf = np.ascontiguousarray(
        hT.astype(BF_NP).reshape(KT, 128, B).transpose(1, 0, 2)
    )

    # gather caches via the block table (b-major), slice heads per core
    k_all = k_cache[block_offsets.reshape(-1)]                        # [512,64,32,128]
    v_all = v_cache[block_offsets.reshape(-1)]

    ident = np.eye(B, dtype=np.float32)

    in_maps = []
    for c in range(NCORES):
        h0 = c * HL
        qcols = np.arange(h0 * D, (h0 + HL) * D)
        wp_c = np.concatenate(
            [W_pack[:, qcols], W_pack[:, HID + qcols], W_pack[:, 2 * HID + qcols]],
            axis=1,
        ).astype(BF_NP)                                               # [4096, 1536]
        wp_c = np.ascontiguousarray(
            wp_c.reshape(KT, 128, 3 * HL * D).transpose(1, 0, 2)
        )                                                             # [128,KT,1536]

        wo_c = np.ascontiguousarray(o_proj_weight[:, qcols].T).astype(BF_NP)
        wo_c = wo_c.reshape(HL, 128, HID)                             # [4,128,4096]

        # pack per-request KV into one contiguous strip:
        # [K: (h, s16-exact) | V: (pair, h, d) 128-padded]
        kv_pk = np.zeros((128, max(G, 1)), BF_NP)
        for b in range(B):
            pb = pairs[b]
            if pb == 0:
                continue
            s16 = s16s[b]
            blk = k_all[b * NBLK : b * NBLK + 2 * pb, :, h0 : h0 + HL, :]
            kb = (blk.reshape(pb, 128, HL, D).transpose(3, 2, 0, 1)
                  .reshape(128, HL, pb * 128)[:, :, 0:s16]
                  .reshape(128, HL * s16))                            # [d,(h,s16)]
            blk = v_all[b * NBLK : b * NBLK + 2 * pb, :, h0 : h0 + HL, :]
            vb = (blk.reshape(pb, 128, HL, D).transpose(1, 0, 2, 3)
                  .reshape(128, pb * 512))                            # [s,(p,h,d)]
            kv_pk[:, offs[b] : offs[b] + HL * s16] = kb
            kv_pk[:, offs[b] + HL * s16 : offs[b + 1]] = vb

        in_maps.append({
            "hT": hT_bf, "wp": wp_c, "wo": wo_c, "kv": kv_pk,
            "cs": cs, "mask": mask, "ident": ident,
        })
    return (pairs, s16s), in_maps


def kernel(hidden_states, W_pack, o_proj_weight, k_cache, v_cache,
           history_lengths, block_offsets):
    global LAST_RESULTS
    pairs, in_maps = _host_prep(
        hidden_states, W_pack, o_proj_weight, k_cache, v_cache,
        history_lengths, block_offsets,
    )
    nc = _build_nc(pairs)
    trace = bool(int(os.environ.get("KERNEL_TRACE", "0")))
    res = run_bass_kernel_spmd(nc, in_maps, list(range(NCORES)), trace=trace)
    LAST_RESULTS = res
    out = np.zeros((B, HID), np.float32)
    for c in range(NCORES):
        out += res.results[c]["out_part"]
    return out


# revision 24
# speedup vs baseline: 1.1966x; 1.1966x over previous
"""Paged decode attention (nn_Attention_5626407157951) on 8 Trainium2 cores.

Tensor-parallel over heads: each core owns 4 of 32 heads. Per core:
  qkv = hidden @ W_pack[:, own cols]      (bf16 matmuls, fp32 acc)
  rotary(q, k) at pos=hist                (DVE, fp32; host-built cos/sin)
  scores_T[s, (h,pair)] = K_tile^T q      (PE, K stationary, q moving, bf16)
  softmax without max-subtraction; new token handled analytically:
      out = (sum_s exp(s)*v_s + e_new*v_new) / (sum_s exp(s) + e_new)
  out_partial = attn @ o_proj[:, own dims].T ; host sums the 8 partials.

Everything DMA'd is bf16 (tolerance is 2e-2; bf16 end-to-end lands ~1e-3).
KV is host-packed per request (only valid 128-token pairs), contiguous in
DRAM so each request is one large DMA with multi-KB per-partition runs.
"""

import math
import os

import ml_dtypes
import numpy as np

import concourse.bass as bass
import concourse.mybir as mybir
import concourse.tile as tile
from concourse.bass_utils import run_bass_kernel_spmd
from concourse.vector_clock import ScopedClock

B = 32          # batch (decode requests)
H = 32          # total heads
HL = 4          # heads per core
D = 128         # head dim
HID = 4096
BS = 64         # cache block size
NBLK = 16       # blocks per request
NCORES = 8
KT = HID // 128         # 32 contraction tiles for qkv proj
PAIRS = NBLK // 2       # 8 block-pairs (128 tokens each) per request
ROPE_BASE = 10000.0
KTB = 2                 # W_pack kt tiles fetched per DMA

F32 = mybir.dt.float32
BF = mybir.dt.bfloat16
BF_NP = ml_dtypes.bfloat16
EXP_FN = mybir.ActivationFunctionType.Exp
MUL = mybir.AluOpType.mult
ADD = mybir.AluOpType.add
SUB = mybir.AluOpType.subtract

LAST_RESULTS = None  # test harness peeks at this for profiling info

# ---------------------------------------------------------------------------
# This walrus build accepts very few sync-waits per instruction; the Tile
# kernel-tail drain accumulates one wait per sem lane. Split the waits over
# several drain instructions (all before the barrier, so semantics hold).
_MAX_DRAIN_WAITS = 1


def _patched_drain_and_barrier(self, tick_clock, wait_clock):
    nc = self.nc
    drain_inst = nc.sync.drain()
    wait_clock.add_sem_waits(
        drain_inst.ins, ScopedClock({None: tick_clock.global_clock})
    )
    si = drain_inst.ins.sync_info
    if si is not None and si.on_wait and len(si.on_wait) > _MAX_DRAIN_WAITS:
        waits = list(si.on_wait)
        drain_inst.ins.sync_info = mybir.SyncInfo(
            on_wait=waits[:_MAX_DRAIN_WAITS], on_update=list(si.on_update or [])
        )
        rest = waits[_MAX_DRAIN_WAITS:]
        for i in range(0, len(rest), _MAX_DRAIN_WAITS):
            extra = nc.sync.drain()
            extra.ins.sync_info = mybir.SyncInfo(
                on_wait=rest[i : i + _MAX_DRAIN_WAITS], on_update=[]
            )
    nc.all_engine_barrier()
    popped = nc._tile_sem_poison_stack.pop()
    assert popped is self._sem_poison
    nc.clear_and_free_semaphores(list(self.sems.allocated().values()))
    nc.all_engine_barrier()


tile.TileContext._drain_and_barrier = _patched_drain_and_barrier


def _split_excess_waits(nc, limit=1):
    """Walrus rejects instructions carrying more than ~1 sync wait. Hoist the
    excess onto NoOps inserted just before, on the same engine queue (the
    queue blocks on them first, so semantics are identical)."""
    for fn in nc.m.functions:
        for bb in fn.blocks:
            out = []
            changed = False
            for inst in list(bb.instructions):
                si = getattr(inst, "sync_info", None)
                if si is not None and si.on_wait and len(si.on_wait) > limit:
                    waits = list(si.on_wait)
                    extra, keep = waits[:-limit], waits[-limit:]
                    for i in range(0, len(extra), limit):
                        nop = mybir.InstNoOp(
                            name=nc.get_next_instruction_name(),
                            ins=[], outs=[], engine=inst.engine,
                            sync_info=mybir.SyncInfo(
                                on_wait=extra[i : i + limit], on_update=[]
                            ),
                        )
                        nc.register_instruction(nop)
                        out.append(nop)
                    inst.sync_info = mybir.SyncInfo(
                        on_wait=keep, on_update=list(si.on_update or [])
                    )
                    changed = True
                out.append(inst)
            if changed:
                bb.instructions = out
# ---------------------------------------------------------------------------


def _build_nc(pairs):
    """Build the SPMD bass module. `pairs[b]` = number of 128-token cached
    pairs for request b (same on every core; head split is via input data)."""
    nc = bass.Bass()

    pairs, s16s = pairs
    kcols = [HL * s for s in s16s]            # exact-packed K cols per request
    vcols = [p * 512 for p in pairs]          # 128-padded V cols per request
    offs = np.concatenate([[0], np.cumsum([k + v for k, v in zip(kcols, vcols)])])
    G = int(offs[-1])  # total packed KV columns (per 128-partition row)

    def param(name, shape, dt):
        return nc.declare_dram_parameter(name, list(shape), dt, isOutput=False)

    hT = param("hT", [128, KT, B], BF)
    wp = param("wp", [128, KT, 3 * HL * D], BF)
    wo = param("wo", [HL, 128, HID], BF)
    kv = param("kv", [128, max(G, 1)], BF)   # per request: [K (h,s16) | V (p,h,d)]
    cs = param("cs", [B, 4 * HL * D], F32)
    maskp = param("mask", [128, B, HL, PAIRS], BF)   # multiplicative 0/1
    identp = param("ident", [B, B], F32)
    out_part = nc.declare_dram_parameter("out_part", [B, HID], F32, isOutput=True)

    HD = HL * D  # 512 local attention dims

    with tile.TileContext(nc) as tc:
        with (
            tc.tile_pool(name="const", bufs=1) as cpool,
            tc.tile_pool(name="work", bufs=1) as wpool,
            tc.tile_pool(name="wtiles", bufs=8) as wtp,
            tc.tile_pool(name="kv", bufs=7) as kvp,
            tc.tile_pool(name="small", bufs=4) as smp,
        ):
            # ---- constants ----
            ident = cpool.tile([B, B], F32)
            nc.scalar.dma_start(out=ident[:], in_=identp[:])
            ones = cpool.tile([128, 1], BF)
            nc.vector.memset(ones[:], 1.0)
            onesf = cpool.tile([1, HL * B], F32)
            nc.vector.memset(onesf[:], 1.0)
            mask_sb = cpool.tile([128, B, HL, PAIRS], BF)
            nc.scalar.dma_start(out=mask_sb[:], in_=maskp[:])
            cs_sb = cpool.tile([B, 4 * HD], F32)
            nc.scalar.dma_start(out=cs_sb[:], in_=cs[:])
            hT_sb = cpool.tile([128, KT, B], BF)
            nc.scalar.dma_start(out=hT_sb[:], in_=hT[:])

            # W_pack chunks issued ahead of the KV prefetch (same queue) so
            # the wire strictly prioritizes the qkv phase's critical input;
            # tiny leading chunks let the first matmuls start sooner
            wp_chunks = [(0, 1), (1, 2)] + [(k, k + 2) for k in range(2, KT, 2)]
            wp_tiles = {}

            def load_wp(c):
                a, e = wp_chunks[c]
                wpt = wtp.tile([128, KTB, 3 * HD], BF, tag="wpt")
                nc.sync.dma_start(out=wpt[:, 0 : e - a, :], in_=wp[:, a:e, :])
                wp_tiles[c] = wpt

            for c in range(len(wp_chunks)):
                load_wp(c)

            # per-request KV loads (one DMA per tensor per request)
            kv_tiles = {}

            def load_b(b):
                o = int(offs[b])
                w = int(offs[b + 1]) - o
                t = kvp.tile([128, w], BF, tag="kv")
                nc.sync.dma_start(out=t[:], in_=kv[:, o : o + w])
                kv_tiles[b] = t

            for b in sorted(range(B), key=lambda b: -pairs[b]):
                if pairs[b] > 0:
                    load_b(b)
                    if len(kv_tiles) >= 7:
                        break

            # accumulators written per-b, read in the epilogue
            atsb = wpool.tile([128, HL * B], F32)   # cached attn, col h*32+b
            nc.vector.memset(atsb[:], 0.0)
            dnm = wpool.tile([1, HL * B], F32)      # cached denom, col h*32+b
            nc.vector.memset(dnm[:], 0.0)

            with tc.tile_pool(name="psA", bufs=1, space="PSUM") as psA:
                # PE warmup transpose so `ident` is observed by PE before the
                # real (fp32, single-wait-slot) transposes below.
                tp0 = psA.tile([B, B], F32, tag="tp0")
                nc.tensor.transpose(tp0[:], ident[:], ident[:])

                # ---- phase 1: qkv = hidden @ W_pack (bf16) ----
                with nc.named_scope("qkv"):
                    qkv_ps = psA.tile([B, 3 * HD], F32, tag="qkv")
                    for c, (a, e) in enumerate(wp_chunks):
                        wpt = wp_tiles.pop(c)
                        for kj in range(e - a):
                            kt = a + kj
                            for n in range(3):
                                nc.tensor.matmul(
                                    qkv_ps[:, n * HD : (n + 1) * HD],
                                    hT_sb[:, kt, :],
                                    wpt[:, kj, n * HD : (n + 1) * HD],
                                    start=(kt == 0),
                                    stop=(kt == KT - 1),
                                )

                    qkv_sb = wpool.tile([B, 3 * HD], F32)
                    nc.vector.tensor_copy(qkv_sb[:], qkv_ps[:])

                # ---- phase 2: rotary (fp32, DVE) + transposes ----
                with nc.named_scope("rope"):
                    def rope(src_off, cs_off):
                        src = qkv_sb[:, src_off : src_off + HD]
                        t1 = wpool.tile([B, HD], F32, tag="rope_t1")
                        nc.vector.tensor_tensor(
                            t1[:], src, cs_sb[:, cs_off : cs_off + HD], MUL
                        )
                        sh = wpool.tile([B, HD], F32, tag="rope_sh")
                        sh4 = sh[:].rearrange("b (h d) -> b h d", h=HL)
                        sr4 = qkv_sb[:, src_off : src_off + HD].rearrange(
                            "b (h d) -> b h d", h=HL
                        )
                        cs2 = cs_sb[:, cs_off + HD : cs_off + 2 * HD].rearrange(
                            "b (h d) -> b h d", h=HL
                        )
                        nc.vector.tensor_tensor(
                            sh4[:, :, 0:64], sr4[:, :, 64:128], cs2[:, :, 0:64], MUL
                        )
                        nc.vector.tensor_tensor(
                            sh4[:, :, 64:128], sr4[:, :, 0:64], cs2[:, :, 64:128], MUL
                        )
                        nc.vector.tensor_tensor(
                            qkv_sb[:, src_off : src_off + HD], t1[:], sh[:], ADD
                        )

                    rope(0, 0)          # q (scale folded into tables)
                    rope(HD, 2 * HD)    # k

                    # PE transposes -> [128(d), (h,b)] fp32 tiles
                    qT = wpool.tile([128, HL * B], F32)
                    kT = wpool.tile([128, HL * B], F32)
                    vT = wpool.tile([128, HL * B], F32)
                    for off, dst in ((0, qT), (HD, kT), (2 * HD, vT)):
                        for h in range(HL):
                            tp = psA.tile([128, B], F32, tag="tp")
                            inp = qkv_sb[:, off + h * D : off + (h + 1) * D]
                            nc.tensor.transpose(tp[:], inp, ident[:])
                            nc.vector.tensor_copy(dst[:, h * B : (h + 1) * B], tp[:])

                    qT_bf = wpool.tile([128, HL * B], BF)
                    nc.vector.tensor_copy(qT_bf[:], qT[:])

                    # new-token scores: e_new[(h,b)] = exp(q . k_new)
                    prod = wpool.tile([128, HL * B], F32)
                    nc.vector.tensor_tensor(prod[:], qT[:], kT[:], MUL)
                    prod_bf = wpool.tile([128, HL * B], BF)
                    nc.vector.tensor_copy(prod_bf[:], prod[:])
                    sn_ps = psA.tile([1, HL * B], F32, tag="sn")
                    nc.tensor.matmul(sn_ps[:], ones[:], prod_bf[:], start=True, stop=True)
                    e_new = wpool.tile([1, HL * B], F32)
                    nc.scalar.activation(e_new[:], sn_ps[:], EXP_FN)

            # ---- phase 3: per-request paged attention ----
            # o_proj weight DMAs are interleaved into the attention tail so
            # they fill the wire without delaying critical-path KV loads
            wo_tiles = {}
            border = sorted(range(B), key=lambda b: -pairs[b])
            wo_sched = {24: 0, 27: 1, 29: 2, 31: 3}   # loop positions

            def issue_wo(h):
                woh = kvp.tile([128, HID], BF, tag="kv")
                nc.scalar.dma_start(out=woh[:], in_=wo[h])
                wo_tiles[h] = woh

            with (
                tc.tile_pool(name="psB", bufs=4, space="PSUM") as psB,
                tc.tile_pool(name="psBa", bufs=3, space="PSUM") as psBa,
                tc.tile_pool(name="psB2", bufs=1, space="PSUM") as psB2,
                nc.named_scope("attn"),
            ):
                for bi, b in enumerate(border):
                    if bi in wo_sched:
                        issue_wo(wo_sched[bi])
                    pb = pairs[b]
                    if pb == 0:
                        continue
                    if b not in kv_tiles:
                        load_b(b)
                    ni = bi + 1
                    loaded = sum(1 for t in kv_tiles if t != b)
                    while ni < B and loaded < 6:
                        nb = border[ni]
                        if pairs[nb] > 0 and nb not in kv_tiles:
                            load_b(nb)
                            loaded += 1
                        ni += 1
                    t = kv_tiles.pop(b)
                    s16 = s16s[b]
                    kt_b = t[:, 0 : HL * s16].rearrange("d (h s) -> d h s", h=HL)
                    vt_b = t[:, HL * s16 : HL * s16 + pb * 512].rearrange(
                        "s (p h d) -> s p h d", p=pb, h=HL
                    )

                    # scores^T: [128(s), (h, pair)]; tail rows beyond the
                    # packed K are memset to 0 (exp->1, masked to 0)
                    scp = psB.tile([128, HL, pb], F32, tag="scp")
                    wlast = s16 - (pb - 1) * 128
                    if wlast < 128:
                        nc.vector.memset(scp[:, :, pb - 1], 0.0)
                    for h in range(HL):
                        qh = qT_bf[:, h * B + b : h * B + b + 1]
                        for p in range(pb):
                            w = min(128, s16 - p * 128)
                            nc.tensor.matmul(
                                scp[0:w, h, p : p + 1],
                                kt_b[:, h, p * 128 : p * 128 + w],
                                qh, start=True, stop=True,
                            )

                    # exp -> probs, multiplicative 0/1 mask folded into the
                    # bf16 downcast (invalid slots in the last pair -> 0)
                    expb = smp.tile([128, HL, pb], F32, tag="expb")
                    nc.scalar.activation(expb[:], scp[:], EXP_FN)
                    ph = smp.tile([128, HL, pb], BF, tag="ph")
                    nc.vector.tensor_tensor(
                        ph[:], expb[:], mask_sb[:, b, :, 0:pb], MUL
                    )

                    # attn^T[d, h] = sum_s p[s] * V[s, d]
                    atp = psBa.tile([128, HL], F32, tag="atp")
                    for h in range(HL):
                        for p in range(pb):
                            nc.tensor.matmul(
                                atp[:, h : h + 1],
                                vt_b[:, p, h, :],
                                ph[:, h, p : p + 1],
                                start=(p == 0), stop=(p == pb - 1),
                            )
                    nc.vector.tensor_copy(
                        atsb[:].rearrange("d (h b2) -> d h b2", h=HL)[:, :, b], atp[:]
                    )

                    # denominators: column sums of probs
                    dsp = psB2.tile([1, HL * pb], F32, tag="dsp")
                    nc.tensor.matmul(
                        dsp[:], ones[:], ph[:].rearrange("s h p -> s (h p)"),
                        start=True, stop=True,
                    )
                    nc.vector.reduce_sum(
                        dnm[:].rearrange("o (h b2) -> o h b2", h=HL)[:, :, b],
                        dsp[:].rearrange("o (h p) -> o h p", h=HL),
                        axis=mybir.AxisListType.X,
                    )

            # ---- epilogue: add new token, normalize, project ----
            with nc.named_scope("oproj"):
                dtot = wpool.tile([1, HL * B], F32)
                nc.vector.tensor_tensor(dtot[:], dnm[:], e_new[:], ADD)
                rec = wpool.tile([1, HL * B], F32)
                nc.vector.reciprocal(rec[:], dtot[:])
                att = wpool.tile([128, HL * B], F32)
                with tc.tile_pool(name="psD", bufs=1, space="PSUM") as psD:
                    # broadcast rows across partitions via K=1 outer products
                    ebp = psD.tile([128, HL * B], F32, tag="ebp")
                    nc.tensor.matmul(ebp[:], onesf[:], e_new[:], start=True, stop=True)
                    rbp = psD.tile([128, HL * B], F32, tag="rbp")
                    nc.tensor.matmul(rbp[:], onesf[:], rec[:], start=True, stop=True)

                    nc.vector.tensor_tensor(att[:], vT[:], ebp[:], MUL)
                    nc.vector.tensor_tensor(att[:], att[:], atsb[:], ADD)
                    nc.vector.tensor_tensor(att[:], att[:], rbp[:], MUL)
                att_bf = wpool.tile([128, HL * B], BF)
                nc.vector.tensor_copy(att_bf[:], att[:])

                with tc.tile_pool(name="psC", bufs=3, space="PSUM") as psC:
                    for h in range(HL):
                        if h not in wo_tiles:
                            issue_wo(h)
                    for n in range(8):
                        opsn = psC.tile([B, 512], F32, tag="ops")
                        for h in range(HL):
                            nc.tensor.matmul(
                                opsn[:],
                                att_bf[:, h * B : (h + 1) * B],
                                wo_tiles[h][:, n * 512 : (n + 1) * 512],
                                start=(h == 0),
                                stop=(h == HL - 1),
                            )
                        outc = smp.tile([B, 512], F32, tag="outc")
                        nc.vector.tensor_copy(outc[:], opsn[:])
                        nc.scalar.dma_start(
                            out=out_part[:, n * 512 : (n + 1) * 512], in_=outc[:]
                        )

    _split_excess_waits(nc)
    return nc


def _host_prep(hidden, W_pack, o_proj_weight, k_cache, v_cache, hist, block_offsets):
    """Build the 8 per-core input maps (numpy only)."""
    hidden = np.asarray(hidden, np.float32)
    W_pack = np.asarray(W_pack, np.float32)
    o_proj_weight = np.asarray(o_proj_weight, np.float32)
    k_cache = np.asarray(k_cache, np.float32)
    v_cache = np.asarray(v_cache, np.float32)
    hist = np.asarray(hist, np.int64)
    block_offsets = np.asarray(block_offsets, np.int64)

    pairs = [int((h + 127) // 128) for h in hist]
    s16s = [int((h + 15) // 16 * 16) for h in hist]
    kcols = [HL * s for s in s16s]
    vcols = [p * 512 for p in pairs]
    offs = np.concatenate([[0], np.cumsum([k + v for k, v in zip(kcols, vcols)])])
    G = int(offs[-1])

    # rope tables, scale folded into the q tables
    inv_freq = 1.0 / (ROPE_BASE ** (np.arange(0, D, 2, dtype=np.float32) / D))
    ang = hist.astype(np.float32)[:, None] * inv_freq[None, :]        # [B, 64]
    cos128 = np.concatenate([np.cos(ang), np.cos(ang)], -1)           # [B, 128]
    sin128 = np.concatenate([np.sin(ang), np.sin(ang)], -1)
    sign = np.concatenate([-np.ones(64), np.ones(64)]).astype(np.float32)
    sc = 1.0 / math.sqrt(D)
    tile_h = lambda x: np.tile(x, (1, HL)).astype(np.float32)         # [B, 512]
    cs = np.concatenate(
        [tile_h(cos128 * sc), tile_h(sin128 * sign * sc),
         tile_h(cos128), tile_h(sin128 * sign)], -1,
    )                                                                 # [B, 2048]

    # multiplicative mask over loaded pairs: pos 128*p + s valid iff < hist
    s_idx = np.arange(128)[:, None, None]                             # s
    p_idx = np.arange(PAIRS)[None, None, :]                           # pair
    pos = p_idx * 128 + s_idx                                         # [128,1,8]
    valid = pos < hist[None, :, None]                                 # [128,B,8]
    mask = np.repeat(valid[:, :, None, :], HL, axis=2).astype(BF_NP)  # [128,B,4,8]

    hT = np.ascontiguousarray(hidden.T)                               # [4096, 32]
    hT_bf = np.ascontiguousarray(
        hT.astype(BF_NP).reshape(KT, 128, B).transpose(1, 0, 2)
    )

    # gather caches via the block table (b-major), slice heads per core
    k_all = k_cache[block_offsets.reshape(-1)]                        # [512,64,32,128]
    v_all = v_cache[block_offsets.reshape(-1)]

    ident = np.eye(B, dtype=np.float32)

    in_maps = []
    for c in range(NCORES):
        h0 = c * HL
        qcols = np.arange(h0 * D, (h0 + HL) * D)
        wp_c = np.concatenate(
            [W_pack[:, qcols], W_pack[:, HID + qcols], W_pack[:, 2 * HID + qcols]],
            axis=1,
        ).astype(BF_NP)                                               # [4096, 1536]
        wp_c = np.ascontiguousarray(
            wp_c.reshape(KT, 128, 3 * HL * D).transpose(1, 0, 2)
        )                                                             # [128,KT,1536]

        wo_c = np.ascontiguousarray(o_proj_weight[:, qcols].T).astype(BF_NP)
        wo_c = wo_c.reshape(HL, 128, HID)                             # [4,128,4096]

        # pack per-request KV into one contiguous strip:
        # [K: (h, s16-exact) | V: (pair, h, d) 128-padded]
        kv_pk = np.zeros((128, max(G, 1)), BF_NP)
        for b in range(B):
            pb = pairs[b]
            if pb == 0:
                continue
            s16 = s16s[b]
            blk = k_all[b * NBLK : b * NBLK + 2 * pb, :, h0 : h0 + HL, :]
            kb = (blk.reshape(pb, 128, HL, D).transpose(3, 2, 0, 1)
                  .reshape(128, HL, pb * 128)[:, :, 0:s16]
                  .reshape(128, HL * s16))                            # [d,(h,s16)]
            blk = v_all[b * NBLK : b * NBLK + 2 * pb, :, h0 : h0 + HL, :]
            vb = (blk.reshape(pb, 128, HL, D).transpose(1, 0, 2, 3)
                  .reshape(128, pb * 512))                            # [s,(p,h,d)]
            kv_pk[:, offs[b] : offs[b] + HL * s16] = kb
            kv_pk[:, offs[b] + HL * s16 : offs[b + 1]] = vb

        in_maps.append({
            "hT": hT_bf, "wp": wp_c, "wo": wo_c, "kv": kv_pk,
            "cs": cs, "mask": mask, "ident": ident,
        })
    return (pairs, s16s), in_maps


def kernel(hidden_states, W_pack, o_proj_weight, k_cache, v_cache,
           history_lengths, block_offsets):
    global LAST_RESULTS
    pairs, in_maps = _host_prep(
        hidden_states, W_pack, o_proj_weight, k_cache, v_cache,
        history_lengths, block_offsets,
    )
    nc = _build_nc(pairs)
    trace = bool(int(os.environ.get("KERNEL_TRACE", "0")))
    res = run_bass_kernel_spmd(nc, in_maps, list(range(NCORES)), trace=trace)
    LAST_RESULTS = res
    out = np.zeros((B, HID), np.float32)
    for c in range(NCORES):
        out += res.results[c]["out_part"]
    return out
